# revision 27
# baseline (speedup 1.0000x reference)
"""Trainium2 Bass kernel for nn_BERT1E (triplet loss over span-pooled BERT-style block).

Strategy: data-parallel over the pair dimension P=256 across 8 NeuronCores
(32 pairs/core). Weights are replicated per core, transported as bf16
(loss rel-err ~1e-4, far inside tolerance) and streamed into SBUF up
front so DMA runs continuously; activation math stays fp32. The ragged
span gather is done host-side into fixed-shape zero-padded window
buffers (the per-core shard); mean-pool counts and all arithmetic run on
device. Per-core partial losses (sum of relu terms / 256) are summed on
the host.

Self-contained: hardcodes all shapes; only needs the container toolchain
at /opt/trn_rl_repo.
"""
import os
import sys
import types
from contextlib import ExitStack

for _p in ('/opt/trn_rl_repo', '/root/.axon_site'):
    if _p not in sys.path:
        sys.path.insert(0, _p)

import numpy as np
import ml_dtypes
import concourse.bass as bass
import concourse.bacc as bacc
import concourse.tile as tile
from concourse import mybir
from concourse.masks import make_identity
from concourse.bass_utils import run_bass_kernel_spmd

F32 = mybir.dt.float32
BF16 = mybir.dt.bfloat16
I32 = mybir.dt.int32
AF = mybir.ActivationFunctionType
ALU = mybir.AluOpType

B, S, D, P, H, DFF = 32, 512, 768, 256, 12, 3072
DH = D // H
NCORES = 8
PC = P // NCORES          # 32 pairs per core
R = 2 * PC                # 64 query rows per core (p-block then n-block)
EPS_LN = 1e-5
EPS_TRIP = 1e-6
MARGIN = 1.0
KD = D // 128             # 6 contraction chunks for D
KF = DFF // 128           # 24 contraction chunks for DFF

# pooled partition layout: p_emb 0:32, n_emb 32:64, s_emb 64:96, cls 96:128
LAST_RESULTS = None       # set when BASS_KERNEL_TRACE=1 (for test harness)


def _install_trace_shim():
    """Register the NTFF profile hook that this image's antenv lacks."""
    try:
        if 'antenv.axon_hooks' not in sys.modules:
            import antenv
            mod = types.ModuleType('antenv.axon_hooks')
            _h = [None]
            mod.set_axon_ntff_profile_hook = lambda h: _h.__setitem__(0, h)
            mod.get_axon_ntff_profile_hook = lambda: _h[0]
            sys.modules['antenv.axon_hooks'] = mod
            antenv.axon_hooks = mod
        import antenv.axon_hooks as ah
        if ah.get_axon_ntff_profile_hook() is None:
            from trn_agent_boot.trn_boot import _ntff_profile_via_ctypes
            ah.set_axon_ntff_profile_hook(
                _ntff_profile_via_ctypes('/opt/axon/libaxon_pjrt.so'))
        return True
    except Exception:
        return False


def _kernel_body(ctx, tc, a, out_ap, W):
    nc = tc.nc

    const = ctx.enter_context(tc.tile_pool(name="const", bufs=1))
    gwp = ctx.enter_context(tc.tile_pool(name="gwp", bufs=3))
    small = ctx.enter_context(tc.tile_pool(name="small", bufs=1))
    acts = ctx.enter_context(tc.tile_pool(name="acts", bufs=1))
    stat = ctx.enter_context(tc.tile_pool(name="stat", bufs=1))
    wqkvop = ctx.enter_context(tc.tile_pool(name="wqkvop", bufs=4 * KD))
    w1p = ctx.enter_context(tc.tile_pool(name="w1p", bufs=KD))
    w2p = ctx.enter_context(tc.tile_pool(name="w2p", bufs=24))
    ppool = ctx.enter_context(tc.tile_pool(name="ppool", bufs=6, space="PSUM"))
    tpsum = ctx.enter_context(tc.tile_pool(name="tpsum", bufs=2, space="PSUM"))

    # ---- constants ----
    ident = const.tile([128, 128], F32)
    make_identity(nc, ident[:])
    ident_bf = const.tile([128, 128], BF16)
    make_identity(nc, ident_bf[:])
    epsv = const.tile([R, 1], F32)
    nc.vector.memset(epsv[:], EPS_LN)
    ones_row = const.tile([1, R], BF16)
    nc.gpsimd.dma_start(ones_row[:], a['ones'][:, :])


    # counts
    sp = small.tile([128, 2], I32, tag="spans")
    nc.sync.dma_start(sp[:], a['spans'][:, :])
    cnt_i = small.tile([128, 1], I32, tag="cnt_i")
    nc.vector.tensor_sub(out=cnt_i[:], in0=sp[:, 1:2], in1=sp[:, 0:1])
    cnt_f = small.tile([128, 1], F32, tag="cnt_f")
    nc.vector.tensor_copy(out=cnt_f[:], in_=cnt_i[:])
    rcnt = small.tile([128, 1], F32, tag="rcnt")
    nc.vector.reciprocal(out=rcnt[:], in_=cnt_f[:])

    bias_sb = {}
    for nm, width in (('bq', D), ('bk', D), ('bv', D), ('bo', D),
                      ('b1', DFF), ('b2', D)):
        t = small.tile([1, width], BF16, tag=f"b_{nm}", name=f"b_{nm}")
        nc.gpsimd.dma_start(t[:], a[nm][:, :])
        bias_sb[nm] = t
    # ---- resident weights (DMA streams up front, bf16) ----
    wq_t, wk_t, wv_t, wo_t = [], [], [], []
    for nm, lst in (('wq', wq_t), ('wk', wk_t), ('wv', wv_t), ('wo', wo_t)):
        for k in range(KD):
            t = wqkvop.tile([128, D], BF16, tag="wqkvo", name=f"{nm}_{k}")
            nc.gpsimd.dma_start(t[:], a[nm][128 * k:128 * (k + 1), :])
            lst.append(t)
    lnb = {}
    for nm in ('ln1g', 'ln1b', 'ln2g', 'ln2b'):
        t = small.tile([R, D], F32, tag=f"ln_{nm}", name=f"ln_{nm}")
        src = bass.AP(tensor=a[nm].tensor, offset=a[nm].offset, ap=[[0, R], [1, D]])
        nc.gpsimd.dma_start(t[:], src)
        lnb[nm] = t

    w1_t = []
    for k in range(KD):
        t = w1p.tile([128, DFF], BF16, tag="w1", name=f"w1_{k}")
        nc.sync.dma_start(t[:], a['w1'][128 * k:128 * (k + 1), :])
        w1_t.append(t)
    w2_t = []
    for k in range(KF):
        t = w2p.tile([128, D], BF16, tag="w2t", name=f"w2_{k}")
        nc.sync.dma_start(t[:], a['w2'][128 * k:128 * (k + 1), :])
        w2_t.append(t)


    # ---- chunked pooling + per-chunk transposes ----
    pooled = acts.tile([128, D], F32)
    qT = stat.tile([128, KD, R], BF16)
    kv_sT = stat.tile([128, KD, R], BF16)
    kv_cT = stat.tile([128, KD, R], BF16)
    for c in range(KD):
        cs = slice(128 * c, 128 * (c + 1))
        gw_c = gwp.tile([128, 128, W], F32, tag="gw", name=f"gw_{c}")
        (nc.sync if c % 2 == 0 else nc.scalar).dma_start(
            gw_c[:], a['gwin'][:, cs, :])
        nc.vector.tensor_reduce(out=pooled[:, cs], in_=gw_c[:, :, :],
                                axis=mybir.AxisListType.X, op=ALU.add)
        nc.vector.tensor_scalar_mul(pooled[:, cs], pooled[:, cs], rcnt[:])
        tpq = tpsum.tile([128, R], F32, tag="tp", name=f"tp_poolq_{c}")
        nc.tensor.matmul(tpq[:], lhsT=pooled[0:R, cs],
                         rhs=ident[0:R, 0:R], is_transpose=True,
                         start=True, stop=True)
        tps = tpsum.tile([128, PC], F32, tag="tp", name=f"tp_pools_{c}")
        nc.tensor.matmul(tps[:], lhsT=pooled[64:96, cs],
                         rhs=ident[64:96, 64:96], is_transpose=True,
                         start=True, stop=True)
        tpc = tpsum.tile([128, PC], F32, tag="tp", name=f"tp_poolc_{c}")
        nc.tensor.matmul(tpc[:], lhsT=pooled[96:128, cs],
                         rhs=ident[96:128, 96:128], is_transpose=True,
                         start=True, stop=True, tile_position=(96, 0))
        nc.vector.tensor_copy(out=qT[:, c, :], in_=tpq[:])
        nc.vector.tensor_copy(out=kv_cT[:, c, 0:PC], in_=tpc[:])
        nc.scalar.copy(out=kv_cT[:, c, PC:R], in_=tpc[:])
        # dsT = sT - clsT (both on d-partitions: no shift needed)
        nc.vector.tensor_sub(out=kv_sT[:, c, 0:PC], in0=tps[:],
                             in1=kv_cT[:, c, 0:PC])
        nc.scalar.copy(out=kv_sT[:, c, PC:R], in_=kv_sT[:, c, 0:PC])

    s_dup = acts.tile([R, D], F32, tag="s_dup")
    nc.sync.dma_start(s_dup[0:PC, :], pooled[64:96, :])
    nc.sync.dma_start(s_dup[PC:R, :], pooled[64:96, :])

    NT = ((0, 512), (512, 256))

    def proj(stT, w_tiles, bname, evict):
        psums = [ppool.tile([R, nsz], F32, tag="ps", name=f"ps{nsz}")
                 for (_, nsz) in NT]
        for k in range(KD):
            for ni, (noff, nsz) in enumerate(NT):
                nc.tensor.matmul(psums[ni][:], lhsT=stT[:, k, :],
                                 rhs=w_tiles[k][:, noff:noff + nsz],
                                 start=(k == 0), stop=(bname is None
                                                       and k == KD - 1))
        for ni, (noff, nsz) in enumerate(NT):
            if bname is not None:
                nc.tensor.matmul(psums[ni][:], lhsT=ones_row[:],
                                 rhs=bias_sb[bname][:, noff:noff + nsz],
                                 start=False, stop=True)
            evict(ni, noff, nsz, psums[ni])

    qh = acts.tile([R, D], F32)
    kdp = acts.tile([R, D], F32, tag="kdp", name="kdp")
    dvp = acts.tile([R, D], F32, tag="dvp", name="dvp")
    vh_c = acts.tile([R, D], F32)

    def mk_copy_evict(dst, engine):
        def ev(ni, noff, nsz, ps):
            if engine == 'v':
                nc.vector.tensor_copy(out=dst[:, noff:noff + nsz], in_=ps[:])
            else:
                nc.scalar.copy(out=dst[:, noff:noff + nsz], in_=ps[:])
        return ev

    proj(qT, wq_t, 'bq', mk_copy_evict(qh, 's'))
    proj(kv_sT, wk_t, None, mk_copy_evict(kdp, 'v'))
    proj(kv_sT, wv_t, None, mk_copy_evict(dvp, 's'))
    proj(kv_cT, wv_t, 'bv', mk_copy_evict(vh_c, 'v'))

    # ---- attention core ----
    qkd = acts.tile([R, D], F32, tag="kd", name="qkd")
    nc.vector.tensor_mul(out=qkd[:], in0=qh[:], in1=kdp[:])
    delta = small.tile([R, H], F32, tag="delta")
    nc.vector.tensor_reduce(out=delta[:], in_=qkd[:].rearrange(
        "p (h d) -> p h d", h=H), axis=mybir.AxisListType.X, op=ALU.add)
    a_s = small.tile([R, H], F32, tag="a_s")
    nc.scalar.activation(a_s[:], delta[:], AF.Sigmoid, scale=1.0 / np.sqrt(DH))

    o = acts.tile([R, H, DH], F32, tag="o")
    for h in range(H):
        nc.vector.scalar_tensor_tensor(
            out=o[:, h, :], in0=dvp[:, DH * h:DH * (h + 1)],
            scalar=a_s[:, h:h + 1],
            in1=vh_c[:, DH * h:DH * (h + 1)],
            op0=ALU.mult, op1=ALU.add)
    o_flat = o[:].rearrange("p h d -> p (h d)")

    oT = stat.tile([128, KD, R], BF16, tag="oT")
    tp_o = tpsum.tile([128, KD * R], F32, tag="tp", name="tp_o")
    for j in range(KD):
        nc.tensor.matmul(tp_o[:, R * j:R * (j + 1)],
                         lhsT=o_flat[:, 128 * j:128 * (j + 1)],
                         rhs=ident[0:R, 0:R], is_transpose=True,
                         start=(j == 0), stop=(j == KD - 1))
    nc.scalar.copy(out=oT[:].rearrange("p a b -> p (a b)"), in_=tp_o[:])

    x1 = acts.tile([R, D], F32, tag="x1")

    def ev_x1(ni, noff, nsz, ps):
        nc.vector.tensor_add(out=x1[:, noff:noff + nsz], in0=ps[:],
                             in1=pooled[0:R, noff:noff + nsz])
    proj(oT, wo_t, 'bo', ev_x1)

    def layer_norm(src, gname, bbname, dst):
        bst = small.tile([R, 3, 6], F32, tag="bst", name="bst")
        for i in range(3):
            nc.vector.bn_stats(out=bst[:, i, :], in_=src[:, 256 * i:256 * (i + 1)])
        mv = small.tile([R, 2], F32, tag="mv", name="mv")
        nc.vector.bn_aggr(out=mv[:], in_=bst[:])
        sd = small.tile([R, 1], F32, tag="sd", name="sd")
        nc.scalar.activation(sd[:], mv[:, 1:2], AF.Sqrt, bias=epsv[:])
        rstd = small.tile([R, 1], F32, tag="rstd", name="rstd")
        nc.vector.reciprocal(out=rstd[:], in_=sd[:])
        tnorm = acts.tile([R, D], F32, tag="tnorm", name="tnorm")
        nc.vector.tensor_scalar(out=tnorm[:], in0=src[:], scalar1=mv[:, 0:1],
                                scalar2=rstd[:], op0=ALU.subtract, op1=ALU.mult)
        nc.vector.tensor_mul(out=tnorm[:], in0=tnorm[:], in1=lnb[gname][:])
        nc.vector.tensor_add(out=dst[:], in0=tnorm[:], in1=lnb[bbname][:])

    hmid = acts.tile([R, D], F32, tag="hmid")
    layer_norm(x1, 'ln1g', 'ln1b', hmid)

    # ---- FFN ----
    hT = stat.tile([128, KD, R], BF16, tag="hT")
    tp_h = tpsum.tile([128, KD * R], F32, tag="tp", name="tp_h")
    for j in range(KD):
        nc.tensor.matmul(tp_h[:, R * j:R * (j + 1)],
                         lhsT=hmid[:, 128 * j:128 * (j + 1)],
                         rhs=ident[0:R, 0:R], is_transpose=True,
                         start=(j == 0), stop=(j == KD - 1))
    nc.vector.tensor_copy(out=hT[:].rearrange("p a b -> p (a b)"), in_=tp_h[:])

    r_act = acts.tile([R, DFF], BF16, tag="r_act")
    ps1 = [ppool.tile([R, 512], F32, tag="ps", name=f"f1_{i}")
           for i in range(6)]
    for k in range(KD):
        for ni in range(6):
            nc.tensor.matmul(ps1[ni][:], lhsT=hT[:, k, :],
                             rhs=w1_t[k][:, 512 * ni:512 * (ni + 1)],
                             start=(k == 0), stop=False)
    for ni in range(6):
        noff = 512 * ni
        nc.tensor.matmul(ps1[ni][:], lhsT=ones_row[:],
                         rhs=bias_sb['b1'][:, noff:noff + 512],
                         start=False, stop=True)
        nc.scalar.activation(r_act[:, noff:noff + 512], ps1[ni][:], AF.Relu)

    rT = stat.tile([128, KF, R], BF16, tag="rT")
    for g in range(3):
        tp_r = tpsum.tile([128, 8 * R], BF16, tag="tp", name=f"tp_r{g}")
        for j in range(8):
            col = 128 * (8 * g + j)
            nc.tensor.matmul(tp_r[:, R * j:R * (j + 1)],
                             lhsT=r_act[:, col:col + 128],
                             rhs=ident_bf[0:R, 0:R],
                             is_transpose=True, start=(j == 0), stop=(j == 7))
        nc.vector.tensor_copy(
            out=rT[:, 8 * g:8 * (g + 1), :].rearrange("p a b -> p (a b)"),
            in_=tp_r[:])

    x2 = acts.tile([R, D], F32, tag="x2")
    ps2 = [ppool.tile([R, nsz], F32, tag="ps", name=f"f2_{nsz}")
           for (_, nsz) in NT]
    for k in range(KF):
        for ni, (noff, nsz) in enumerate(NT):
            nc.tensor.matmul(ps2[ni][:], lhsT=rT[:, k, :],
                             rhs=w2_t[k][:, noff:noff + nsz],
                             start=(k == 0), stop=False)
    for ni, (noff, nsz) in enumerate(NT):
        nc.tensor.matmul(ps2[ni][:], lhsT=ones_row[:],
                         rhs=bias_sb['b2'][:, noff:noff + nsz],
                         start=False, stop=True)
        nc.vector.tensor_add(out=x2[:, noff:noff + nsz], in0=ps2[ni][:],
                             in1=hmid[:, noff:noff + nsz])

    atten = acts.tile([R, D], F32, tag="atten")
    layer_norm(x2, 'ln2g', 'ln2b', atten)

    # ---- triplet ----
    diff = acts.tile([R, D], F32, tag="diff")
    nc.vector.scalar_tensor_tensor(out=diff[:], in0=s_dup[:], scalar=EPS_TRIP,
                                   in1=atten[:], op0=ALU.add, op1=ALU.subtract)
    ssq = small.tile([R, 1], F32, tag="ssq")
    sqscr = acts.tile([R, D], F32, tag="tnorm", name="sqscr")
    nc.scalar.activation(sqscr[:], diff[:], AF.Square, accum_out=ssq[:])
    dpn = small.tile([R, 1], F32, tag="dpn")
    nc.scalar.activation(dpn[:], ssq[:], AF.Sqrt, bias=0.0)

    tp_d = tpsum.tile([1, R], F32, tag="tp", name="tp_d")
    nc.tensor.matmul(tp_d[:], lhsT=dpn[:], rhs=ident[0:R, 0:R],
                     is_transpose=True, start=True, stop=True)
    drow = small.tile([1, R], F32, tag="drow")
    nc.vector.tensor_copy(out=drow[:], in_=tp_d[:])

    terms = small.tile([1, PC], F32, tag="terms")
    nc.vector.scalar_tensor_tensor(out=terms[:], in0=drow[:, 0:PC],
                                   scalar=MARGIN, in1=drow[:, PC:R],
                                   op0=ALU.add, op1=ALU.subtract)
    terms2 = small.tile([1, PC], F32, tag="terms2")
    nc.vector.tensor_scalar(out=terms2[:], in0=terms[:], scalar1=0.0,
                            scalar2=1.0 / P, op0=ALU.max, op1=ALU.mult)
    total = small.tile([1, 1], F32, tag="total")
    nc.vector.tensor_reduce(out=total[:], in_=terms2[:],
                            axis=mybir.AxisListType.X, op=ALU.add)
    nc.sync.dma_start(out_ap[:, :], total[:])


_BUILD_CACHE = {}


def _build(W):
    nc = bacc.Bacc("TRN2", target_bir_lowering=False, debug=False,
                   num_devices=NCORES)
    a = {}

    def din(name, shape, dt=F32):
        a[name] = nc.dram_tensor(name, shape, dt, kind="ExternalInput").ap()

    din('gwin', (128, D, W))
    din('spans', (128, 2), I32)
    din('ones', (1, R), BF16)
    din('wq', (D, D), BF16); din('wk', (D, D), BF16)
    din('wv', (D, D), BF16); din('wo', (D, D), BF16)
    din('w1', (D, DFF), BF16); din('w2', (DFF, D), BF16)
    din('bq', (1, D), BF16); din('bk', (1, D), BF16)
    din('bv', (1, D), BF16); din('bo', (1, D), BF16)
    din('b1', (1, DFF), BF16); din('b2', (1, D), BF16)
    din('ln1g', (1, D)); din('ln1b', (1, D))
    din('ln2g', (1, D)); din('ln2b', (1, D))
    out_ap = nc.dram_tensor('out', (1, 1), F32, kind="ExternalOutput").ap()

    with tile.TileContext(nc) as tc:
        with ExitStack() as ctx:
            _kernel_body(ctx, tc, a, out_ap, W)
    nc.compile()
    return nc, a


def build_in_maps(inputs, W):
    sent = np.ascontiguousarray(np.asarray(inputs['sent_emb'], np.float32))
    sidx = np.asarray(inputs['pair_sidx'])
    spans = {t: np.asarray(inputs[f'{t}_span']) for t in 'spn'}

    def bf(x):
        return np.ascontiguousarray(np.asarray(x, np.float32)).astype(
            ml_dtypes.bfloat16)

    rep = {
        'wq': bf(inputs['Wq']), 'wk': bf(inputs['Wk']),
        'wv': bf(inputs['Wv']), 'wo': bf(inputs['Wo']),
        'w1': bf(inputs['W1']), 'w2': bf(inputs['W2']),
        'bq': bf(inputs['bq']).reshape(1, D), 'bk': bf(inputs['bk']).reshape(1, D),
        'bv': bf(inputs['bv']).reshape(1, D), 'bo': bf(inputs['bo']).reshape(1, D),
        'b1': bf(inputs['b1']).reshape(1, DFF), 'b2': bf(inputs['b2']).reshape(1, D),
        'ones': np.ones((1, R), ml_dtypes.bfloat16),
        'ln1g': np.asarray(inputs['ln1_g'], np.float32).reshape(1, D),
        'ln1b': np.asarray(inputs['ln1_b'], np.float32).reshape(1, D),
        'ln2g': np.asarray(inputs['ln2_g'], np.float32).reshape(1, D),
        'ln2b': np.asarray(inputs['ln2_b'], np.float32).reshape(1, D),
    }

    in_maps = []
    for c in range(NCORES):
        gwin = np.zeros((128, W, D), np.float32)
        spans2 = np.zeros((128, 2), np.int32)
        spans2[96:, 1] = 1  # cls count = 1
        for b, t in enumerate('pns'):  # partition blocks: p, n, s
            sl = spans[t][c * PC:(c + 1) * PC]
            spans2[b * PC:(b + 1) * PC] = sl
            for i in range(PC):
                a0, a1 = int(sl[i, 0]), int(sl[i, 1])
                L = min(a1 - a0, W)
                if L > 0:
                    gwin[b * PC + i, :L] = sent[sidx[c * PC + i], a0:a0 + L]
        for i in range(PC):
            gwin[96 + i, 0] = sent[sidx[c * PC + i], 0]
        m = dict(rep)
        m['gwin'] = np.ascontiguousarray(gwin.transpose(0, 2, 1))
        m['spans'] = spans2
        in_maps.append(m)
    return in_maps


def kernel(**inputs):
    global LAST_RESULTS
    maxlen = max(int(np.max(np.asarray(inputs[f'{t}_span'])[:, 1]
                            - np.asarray(inputs[f'{t}_span'])[:, 0]))
                 for t in 'spn')
    W = max(1, min(maxlen, 64))
    assert maxlen <= 64, f"span length {maxlen} exceeds supported window"

    if W not in _BUILD_CACHE:
        _BUILD_CACHE[W] = _build(W)
    nc, _ = _BUILD_CACHE[W]

    in_maps = build_in_maps(inputs, W)
    trace = bool(os.environ.get('BASS_KERNEL_TRACE'))
    kwargs = {}
    if trace:
        trace = _install_trace_shim()
        if trace and os.environ.get('BASS_KERNEL_TRACE_ALL'):
            kwargs['trace_cores'] = list(range(NCORES))
            kwargs['stitch_traces'] = True
    res = run_bass_kernel_spmd(nc, in_maps, core_ids=list(range(NCORES)),
                               trace=trace, **kwargs)
    if trace:
        LAST_RESULTS = res
    total = sum(float(res.results[c]['out'][0, 0]) for c in range(NCORES))
    return np.array(total, dtype=np.float32)


# revision 28
# speedup vs baseline: 1.4204x; 1.4204x over previous
"""Trainium2 Bass kernel for nn_BERT1E (triplet loss over span-pooled BERT-style block).

Strategy: data-parallel over the pair dimension P=256 across 8 NeuronCores
(32 pairs/core). Weights are replicated per core, transported as bf16
(loss rel-err ~1e-4, far inside tolerance) and streamed into SBUF up
front so DMA runs continuously; activation math stays fp32. The ragged
span gather is done host-side into fixed-shape zero-padded window
buffers (the per-core shard); mean-pool counts and all arithmetic run on
device. Per-core partial losses (sum of relu terms / 256) are summed on
the host.

Self-contained: hardcodes all shapes; only needs the container toolchain
at /opt/trn_rl_repo.
"""
import os
import sys
import types
from contextlib import ExitStack

for _p in ('/opt/trn_rl_repo', '/root/.axon_site'):
    if _p not in sys.path:
        sys.path.insert(0, _p)

import numpy as np
import ml_dtypes
import concourse.bass as bass
import concourse.bacc as bacc
import concourse.tile as tile
from concourse import mybir
from concourse.masks import make_identity
from concourse.bass_utils import run_bass_kernel_spmd

F32 = mybir.dt.float32
BF16 = mybir.dt.bfloat16
I32 = mybir.dt.int32
AF = mybir.ActivationFunctionType
ALU = mybir.AluOpType

B, S, D, P, H, DFF = 32, 512, 768, 256, 12, 3072
DH = D // H
NCORES = 8
PC = P // NCORES          # 32 pairs per core
R = 2 * PC                # 64 query rows per core (p-block then n-block)
EPS_LN = 1e-5
EPS_TRIP = 1e-6
MARGIN = 1.0
KD = D // 128             # 6 contraction chunks for D
KF = DFF // 128           # 24 contraction chunks for DFF

# pooled partition layout: p_emb 0:32, n_emb 32:64, s_emb 64:96, cls 96:128
LAST_RESULTS = None       # set when BASS_KERNEL_TRACE=1 (for test harness)


def _install_trace_shim():
    """Register the NTFF profile hook that this image's antenv lacks."""
    try:
        if 'antenv.axon_hooks' not in sys.modules:
            import antenv
            mod = types.ModuleType('antenv.axon_hooks')
            _h = [None]
            mod.set_axon_ntff_profile_hook = lambda h: _h.__setitem__(0, h)
            mod.get_axon_ntff_profile_hook = lambda: _h[0]
            sys.modules['antenv.axon_hooks'] = mod
            antenv.axon_hooks = mod
        import antenv.axon_hooks as ah
        if ah.get_axon_ntff_profile_hook() is None:
            from trn_agent_boot.trn_boot import _ntff_profile_via_ctypes
            ah.set_axon_ntff_profile_hook(
                _ntff_profile_via_ctypes('/opt/axon/libaxon_pjrt.so'))
        return True
    except Exception:
        return False


def _kernel_body(ctx, tc, a, out_ap, W):
    nc = tc.nc

    const = ctx.enter_context(tc.tile_pool(name="const", bufs=1))
    gwp = ctx.enter_context(tc.tile_pool(name="gwp", bufs=3))
    small = ctx.enter_context(tc.tile_pool(name="small", bufs=1))
    acts = ctx.enter_context(tc.tile_pool(name="acts", bufs=1))
    stat = ctx.enter_context(tc.tile_pool(name="stat", bufs=1))
    wqkvop = ctx.enter_context(tc.tile_pool(name="wqkvop", bufs=4 * KD))
    w1p = ctx.enter_context(tc.tile_pool(name="w1p", bufs=KD))
    w2p = ctx.enter_context(tc.tile_pool(name="w2p", bufs=24))
    ppool = ctx.enter_context(tc.tile_pool(name="ppool", bufs=6, space="PSUM"))
    tpsum = ctx.enter_context(tc.tile_pool(name="tpsum", bufs=2, space="PSUM"))

    # ---- constants ----
    ident = const.tile([128, 128], F32)
    make_identity(nc, ident[:])
    ident_bf = const.tile([128, 128], BF16)
    make_identity(nc, ident_bf[:])
    epsv = const.tile([R, 1], F32)
    nc.vector.memset(epsv[:], EPS_LN)
    ones_row = const.tile([1, R], BF16)
    nc.gpsimd.dma_start(ones_row[:], a['ones'][:, :])


    # counts
    sp = small.tile([128, 2], I32, tag="spans")
    nc.sync.dma_start(sp[:], a['spans'][:, :])
    cnt_i = small.tile([128, 1], I32, tag="cnt_i")
    nc.vector.tensor_sub(out=cnt_i[:], in0=sp[:, 1:2], in1=sp[:, 0:1])
    cnt_f = small.tile([128, 1], F32, tag="cnt_f")
    nc.vector.tensor_copy(out=cnt_f[:], in_=cnt_i[:])
    rcnt = small.tile([128, 1], F32, tag="rcnt")
    nc.vector.reciprocal(out=rcnt[:], in_=cnt_f[:])

    bias_sb = {}
    for nm, width in (('bq', D), ('bk', D), ('bv', D), ('bo', D),
                      ('b1', DFF), ('b2', D)):
        t = small.tile([1, width], BF16, tag=f"b_{nm}", name=f"b_{nm}")
        nc.gpsimd.dma_start(t[:], a[nm][:, :])
        bias_sb[nm] = t
    # ---- resident weights (DMA streams up front, bf16) ----
    wq_t, wk_t, wv_t, wo_t = [], [], [], []
    for nm, lst in (('wq', wq_t), ('wk', wk_t), ('wv', wv_t), ('wo', wo_t)):
        for k in range(KD):
            t = wqkvop.tile([128, D], BF16, tag="wqkvo", name=f"{nm}_{k}")
            nc.gpsimd.dma_start(t[:], a[nm][128 * k:128 * (k + 1), :])
            lst.append(t)
    lnb = {}
    for nm in ('ln1g', 'ln1b', 'ln2g', 'ln2b'):
        t = small.tile([R, D], F32, tag=f"ln_{nm}", name=f"ln_{nm}")
        src = bass.AP(tensor=a[nm].tensor, offset=a[nm].offset, ap=[[0, R], [1, D]])
        nc.gpsimd.dma_start(t[:], src)
        lnb[nm] = t

    w1_t = []
    for k in range(KD):
        t = w1p.tile([128, DFF], BF16, tag="w1", name=f"w1_{k}")
        nc.gpsimd.dma_start(t[:], a['w1'][128 * k:128 * (k + 1), :])
        w1_t.append(t)
    w2_t = []
    for k in range(KF):
        t = w2p.tile([128, D], BF16, tag="w2t", name=f"w2_{k}")
        nc.gpsimd.dma_start(t[:], a['w2'][128 * k:128 * (k + 1), :])
        w2_t.append(t)


    # ---- chunked pooling + per-chunk transposes ----
    pooled = acts.tile([128, D], F32)
    qT = stat.tile([128, KD, R], BF16)
    kv_sT = stat.tile([128, KD, R], BF16)
    kv_cT = stat.tile([128, KD, R], BF16)
    for c in range(KD):
        cs = slice(128 * c, 128 * (c + 1))
        gw_c = gwp.tile([128, 128, W], F32, tag="gw", name=f"gw_{c}")
        (nc.sync if c % 2 == 0 else nc.scalar).dma_start(
            gw_c[:], a['gwin'][:, cs, :])
        nc.vector.tensor_reduce(out=pooled[:, cs], in_=gw_c[:, :, :],
                                axis=mybir.AxisListType.X, op=ALU.add)
        nc.vector.tensor_scalar_mul(pooled[:, cs], pooled[:, cs], rcnt[:])
        tpq = tpsum.tile([128, R], F32, tag="tp", name=f"tp_poolq_{c}")
        nc.tensor.matmul(tpq[:], lhsT=pooled[0:R, cs],
                         rhs=ident[0:R, 0:R], is_transpose=True,
                         start=True, stop=True)
        tps = tpsum.tile([128, PC], F32, tag="tp", name=f"tp_pools_{c}")
        nc.tensor.matmul(tps[:], lhsT=pooled[64:96, cs],
                         rhs=ident[64:96, 64:96], is_transpose=True,
                         start=True, stop=True)
        tpc = tpsum.tile([128, PC], F32, tag="tp", name=f"tp_poolc_{c}")
        nc.tensor.matmul(tpc[:], lhsT=pooled[96:128, cs],
                         rhs=ident[96:128, 96:128], is_transpose=True,
                         start=True, stop=True, tile_position=(96, 0))
        nc.vector.tensor_copy(out=qT[:, c, :], in_=tpq[:])
        nc.vector.tensor_copy(out=kv_cT[:, c, 0:PC], in_=tpc[:])
        nc.scalar.copy(out=kv_cT[:, c, PC:R], in_=tpc[:])
        # dsT = sT - clsT (both on d-partitions: no shift needed)
        nc.vector.tensor_sub(out=kv_sT[:, c, 0:PC], in0=tps[:],
                             in1=kv_cT[:, c, 0:PC])
        nc.scalar.copy(out=kv_sT[:, c, PC:R], in_=kv_sT[:, c, 0:PC])

    s_dup = acts.tile([R, D], F32, tag="s_dup")
    nc.sync.dma_start(s_dup[0:PC, :], pooled[64:96, :])
    nc.sync.dma_start(s_dup[PC:R, :], pooled[64:96, :])

    NT = ((0, 512), (512, 256))

    def proj(stT, w_tiles, bname, evict):
        psums = [ppool.tile([R, nsz], F32, tag="ps", name=f"ps{nsz}")
                 for (_, nsz) in NT]
        for k in range(KD):
            for ni, (noff, nsz) in enumerate(NT):
                nc.tensor.matmul(psums[ni][:], lhsT=stT[:, k, :],
                                 rhs=w_tiles[k][:, noff:noff + nsz],
                                 start=(k == 0), stop=(bname is None
                                                       and k == KD - 1))
        for ni, (noff, nsz) in enumerate(NT):
            if bname is not None:
                nc.tensor.matmul(psums[ni][:], lhsT=ones_row[:],
                                 rhs=bias_sb[bname][:, noff:noff + nsz],
                                 start=False, stop=True)
            evict(ni, noff, nsz, psums[ni])

    qh = acts.tile([R, D], F32)
    kdp = acts.tile([R, D], F32, tag="kdp", name="kdp")
    dvp = acts.tile([R, D], F32, tag="dvp", name="dvp")
    vh_c = acts.tile([R, D], F32)

    def mk_copy_evict(dst, engine):
        def ev(ni, noff, nsz, ps):
            if engine == 'v':
                nc.vector.tensor_copy(out=dst[:, noff:noff + nsz], in_=ps[:])
            else:
                nc.scalar.copy(out=dst[:, noff:noff + nsz], in_=ps[:])
        return ev

    proj(qT, wq_t, 'bq', mk_copy_evict(qh, 's'))
    proj(kv_sT, wk_t, None, mk_copy_evict(kdp, 'v'))
    proj(kv_sT, wv_t, None, mk_copy_evict(dvp, 's'))
    proj(kv_cT, wv_t, 'bv', mk_copy_evict(vh_c, 'v'))

    # ---- attention core ----
    qkd = acts.tile([R, D], F32, tag="kd", name="qkd")
    nc.vector.tensor_mul(out=qkd[:], in0=qh[:], in1=kdp[:])
    delta = small.tile([R, H], F32, tag="delta")
    nc.vector.tensor_reduce(out=delta[:], in_=qkd[:].rearrange(
        "p (h d) -> p h d", h=H), axis=mybir.AxisListType.X, op=ALU.add)
    a_s = small.tile([R, H], F32, tag="a_s")
    nc.scalar.activation(a_s[:], delta[:], AF.Sigmoid, scale=1.0 / np.sqrt(DH))

    o = acts.tile([R, H, DH], F32, tag="o")
    for h in range(H):
        nc.vector.scalar_tensor_tensor(
            out=o[:, h, :], in0=dvp[:, DH * h:DH * (h + 1)],
            scalar=a_s[:, h:h + 1],
            in1=vh_c[:, DH * h:DH * (h + 1)],
            op0=ALU.mult, op1=ALU.add)
    o_flat = o[:].rearrange("p h d -> p (h d)")

    oT = stat.tile([128, KD, R], BF16, tag="oT")
    tp_o = tpsum.tile([128, KD * R], F32, tag="tp", name="tp_o")
    for j in range(KD):
        nc.tensor.matmul(tp_o[:, R * j:R * (j + 1)],
                         lhsT=o_flat[:, 128 * j:128 * (j + 1)],
                         rhs=ident[0:R, 0:R], is_transpose=True,
                         start=(j == 0), stop=(j == KD - 1))
    nc.scalar.copy(out=oT[:].rearrange("p a b -> p (a b)"), in_=tp_o[:])

    x1 = acts.tile([R, D], F32, tag="x1")

    def ev_x1(ni, noff, nsz, ps):
        nc.vector.tensor_add(out=x1[:, noff:noff + nsz], in0=ps[:],
                             in1=pooled[0:R, noff:noff + nsz])
    proj(oT, wo_t, 'bo', ev_x1)

    def layer_norm(src, gname, bbname, dst):
        bst = small.tile([R, 3, 6], F32, tag="bst", name="bst")
        for i in range(3):
            nc.vector.bn_stats(out=bst[:, i, :], in_=src[:, 256 * i:256 * (i + 1)])
        mv = small.tile([R, 2], F32, tag="mv", name="mv")
        nc.vector.bn_aggr(out=mv[:], in_=bst[:])
        sd = small.tile([R, 1], F32, tag="sd", name="sd")
        nc.scalar.activation(sd[:], mv[:, 1:2], AF.Sqrt, bias=epsv[:])
        rstd = small.tile([R, 1], F32, tag="rstd", name="rstd")
        nc.vector.reciprocal(out=rstd[:], in_=sd[:])
        tnorm = acts.tile([R, D], F32, tag="tnorm", name="tnorm")
        nc.vector.tensor_scalar(out=tnorm[:], in0=src[:], scalar1=mv[:, 0:1],
                                scalar2=rstd[:], op0=ALU.subtract, op1=ALU.mult)
        nc.vector.tensor_mul(out=tnorm[:], in0=tnorm[:], in1=lnb[gname][:])
        nc.vector.tensor_add(out=dst[:], in0=tnorm[:], in1=lnb[bbname][:])

    hmid = acts.tile([R, D], F32, tag="hmid")
    layer_norm(x1, 'ln1g', 'ln1b', hmid)

    # ---- FFN ----
    hT = stat.tile([128, KD, R], BF16, tag="hT")
    tp_h = tpsum.tile([128, KD * R], F32, tag="tp", name="tp_h")
    for j in range(KD):
        nc.tensor.matmul(tp_h[:, R * j:R * (j + 1)],
                         lhsT=hmid[:, 128 * j:128 * (j + 1)],
                         rhs=ident[0:R, 0:R], is_transpose=True,
                         start=(j == 0), stop=(j == KD - 1))
    nc.vector.tensor_copy(out=hT[:].rearrange("p a b -> p (a b)"), in_=tp_h[:])

    r_act = acts.tile([R, DFF], BF16, tag="r_act")
    for half in range(2):
        ps1 = [ppool.tile([R, 512], F32, tag="ps", name=f"f1_{i}")
               for i in range(3)]
        for k in range(KD):
            for ni in range(3):
                noff = 1536 * half + 512 * ni
                nc.tensor.matmul(ps1[ni][:], lhsT=hT[:, k, :],
                                 rhs=w1_t[k][:, noff:noff + 512],
                                 start=(k == 0), stop=False)
        for ni in range(3):
            noff = 1536 * half + 512 * ni
            nc.tensor.matmul(ps1[ni][:], lhsT=ones_row[:],
                             rhs=bias_sb['b1'][:, noff:noff + 512],
                             start=False, stop=True)
            nc.scalar.activation(r_act[:, noff:noff + 512], ps1[ni][:], AF.Relu)

    rT = stat.tile([128, KF, R], BF16, tag="rT")
    for g in range(3):
        tp_r = tpsum.tile([128, 8 * R], BF16, tag="tp", name=f"tp_r{g}")
        for j in range(8):
            col = 128 * (8 * g + j)
            nc.tensor.matmul(tp_r[:, R * j:R * (j + 1)],
                             lhsT=r_act[:, col:col + 128],
                             rhs=ident_bf[0:R, 0:R],
                             is_transpose=True, start=(j == 0), stop=(j == 7))
        nc.vector.tensor_copy(
            out=rT[:, 8 * g:8 * (g + 1), :].rearrange("p a b -> p (a b)"),
            in_=tp_r[:])

    x2 = acts.tile([R, D], F32, tag="x2")
    ps2 = [ppool.tile([R, nsz], F32, tag="ps", name=f"f2_{nsz}")
           for (_, nsz) in NT]
    for k in range(KF):
        for ni, (noff, nsz) in enumerate(NT):
            nc.tensor.matmul(ps2[ni][:], lhsT=rT[:, k, :],
                             rhs=w2_t[k][:, noff:noff + nsz],
                             start=(k == 0), stop=False)
    for ni, (noff, nsz) in enumerate(NT):
        nc.tensor.matmul(ps2[ni][:], lhsT=ones_row[:],
                         rhs=bias_sb['b2'][:, noff:noff + nsz],
                         start=False, stop=True)
        nc.vector.tensor_add(out=x2[:, noff:noff + nsz], in0=ps2[ni][:],
                             in1=hmid[:, noff:noff + nsz])

    atten = acts.tile([R, D], F32, tag="atten")
    layer_norm(x2, 'ln2g', 'ln2b', atten)

    # ---- triplet ----
    diff = acts.tile([R, D], F32, tag="diff")
    nc.vector.scalar_tensor_tensor(out=diff[:], in0=s_dup[:], scalar=EPS_TRIP,
                                   in1=atten[:], op0=ALU.add, op1=ALU.subtract)
    ssq = small.tile([R, 1], F32, tag="ssq")
    sqscr = acts.tile([R, D], F32, tag="tnorm", name="sqscr")
    nc.scalar.activation(sqscr[:], diff[:], AF.Square, accum_out=ssq[:])
    dpn = small.tile([R, 1], F32, tag="dpn")
    nc.scalar.activation(dpn[:], ssq[:], AF.Sqrt, bias=0.0)

    tp_d = tpsum.tile([1, R], F32, tag="tp", name="tp_d")
    nc.tensor.matmul(tp_d[:], lhsT=dpn[:], rhs=ident[0:R, 0:R],
                     is_transpose=True, start=True, stop=True)
    drow = small.tile([1, R], F32, tag="drow")
    nc.vector.tensor_copy(out=drow[:], in_=tp_d[:])

    terms = small.tile([1, PC], F32, tag="terms")
    nc.vector.scalar_tensor_tensor(out=terms[:], in0=drow[:, 0:PC],
                                   scalar=MARGIN, in1=drow[:, PC:R],
                                   op0=ALU.add, op1=ALU.subtract)
    terms2 = small.tile([1, PC], F32, tag="terms2")
    nc.vector.tensor_scalar(out=terms2[:], in0=terms[:], scalar1=0.0,
                            scalar2=1.0 / P, op0=ALU.max, op1=ALU.mult)
    total = small.tile([1, 1], F32, tag="total")
    nc.vector.tensor_reduce(out=total[:], in_=terms2[:],
                            axis=mybir.AxisListType.X, op=ALU.add)
    nc.sync.dma_start(out_ap[:, :], total[:])


_BUILD_CACHE = {}


def _build(W):
    nc = bacc.Bacc("TRN2", target_bir_lowering=False, debug=False,
                   num_devices=NCORES)
    a = {}

    def din(name, shape, dt=F32):
        a[name] = nc.dram_tensor(name, shape, dt, kind="ExternalInput").ap()

    din('gwin', (128, D, W))
    din('spans', (128, 2), I32)
    din('ones', (1, R), BF16)
    din('wq', (D, D), BF16); din('wk', (D, D), BF16)
    din('wv', (D, D), BF16); din('wo', (D, D), BF16)
    din('w1', (D, DFF), BF16); din('w2', (DFF, D), BF16)
    din('bq', (1, D), BF16); din('bk', (1, D), BF16)
    din('bv', (1, D), BF16); din('bo', (1, D), BF16)
    din('b1', (1, DFF), BF16); din('b2', (1, D), BF16)
    din('ln1g', (1, D)); din('ln1b', (1, D))
    din('ln2g', (1, D)); din('ln2b', (1, D))
    out_ap = nc.dram_tensor('out', (1, 1), F32, kind="ExternalOutput").ap()

    with tile.TileContext(nc) as tc:
        with ExitStack() as ctx:
            _kernel_body(ctx, tc, a, out_ap, W)
    nc.compile()
    return nc, a


def build_in_maps(inputs, W):
    sent = np.ascontiguousarray(np.asarray(inputs['sent_emb'], np.float32))
    sidx = np.asarray(inputs['pair_sidx'])
    spans = {t: np.asarray(inputs[f'{t}_span']) for t in 'spn'}

    def bf(x):
        return np.ascontiguousarray(np.asarray(x, np.float32)).astype(
            ml_dtypes.bfloat16)

    rep = {
        'wq': bf(inputs['Wq']), 'wk': bf(inputs['Wk']),
        'wv': bf(inputs['Wv']), 'wo': bf(inputs['Wo']),
        'w1': bf(inputs['W1']), 'w2': bf(inputs['W2']),
        'bq': bf(inputs['bq']).reshape(1, D), 'bk': bf(inputs['bk']).reshape(1, D),
        'bv': bf(inputs['bv']).reshape(1, D), 'bo': bf(inputs['bo']).reshape(1, D),
        'b1': bf(inputs['b1']).reshape(1, DFF), 'b2': bf(inputs['b2']).reshape(1, D),
        'ones': np.ones((1, R), ml_dtypes.bfloat16),
        'ln1g': np.asarray(inputs['ln1_g'], np.float32).reshape(1, D),
        'ln1b': np.asarray(inputs['ln1_b'], np.float32).reshape(1, D),
        'ln2g': np.asarray(inputs['ln2_g'], np.float32).reshape(1, D),
        'ln2b': np.asarray(inputs['ln2_b'], np.float32).reshape(1, D),
    }

    in_maps = []
    for c in range(NCORES):
        gwin = np.zeros((128, W, D), np.float32)
        spans2 = np.zeros((128, 2), np.int32)
        spans2[96:, 1] = 1  # cls count = 1
        for b, t in enumerate('pns'):  # partition blocks: p, n, s
            sl = spans[t][c * PC:(c + 1) * PC]
            spans2[b * PC:(b + 1) * PC] = sl
            for i in range(PC):
                a0, a1 = int(sl[i, 0]), int(sl[i, 1])
                L = min(a1 - a0, W)
                if L > 0:
                    gwin[b * PC + i, :L] = sent[sidx[c * PC + i], a0:a0 + L]
        for i in range(PC):
            gwin[96 + i, 0] = sent[sidx[c * PC + i], 0]
        m = dict(rep)
        m['gwin'] = np.ascontiguousarray(gwin.transpose(0, 2, 1))
        m['spans'] = spans2
        in_maps.append(m)
    return in_maps


def kernel(**inputs):
    global LAST_RESULTS
    maxlen = max(int(np.max(np.asarray(inputs[f'{t}_span'])[:, 1]
                            - np.asarray(inputs[f'{t}_span'])[:, 0]))
                 for t in 'spn')
    W = max(1, min(maxlen, 64))
    assert maxlen <= 64, f"span length {maxlen} exceeds supported window"

    if W not in _BUILD_CACHE:
        _BUILD_CACHE[W] = _build(W)
    nc, _ = _BUILD_CACHE[W]

    in_maps = build_in_maps(inputs, W)
    trace = bool(os.environ.get('BASS_KERNEL_TRACE'))
    kwargs = {}
    if trace:
        trace = _install_trace_shim()
        if trace and os.environ.get('BASS_KERNEL_TRACE_ALL'):
            kwargs['trace_cores'] = list(range(NCORES))
            kwargs['stitch_traces'] = True
    res = run_bass_kernel_spmd(nc, in_maps, core_ids=list(range(NCORES)),
                               trace=trace, **kwargs)
    if trace:
        LAST_RESULTS = res
    total = sum(float(res.results[c]['out'][0, 0]) for c in range(NCORES))
    return np.array(total, dtype=np.float32)
